# revision 1
# baseline (speedup 1.0000x reference)
"""Confusion-matrix kernel v3 for Trainium2 (8 NeuronCores, data-parallel over batch).

Per batch b (one per core):
    pred[n]  = argmax_c input[b, c, n]            (n = pixel, N = H*W)
    cm[i, j] = sum_n target[b, i, n] * (pred[n] == j)
    rs[i]    = sum_n target[b, i, n]
Host: cm_b = cm / (rs + 1e-8); out = mean_b cm_b.

Pixels are slot-packed 6-per-PE-row; a "group" = 128 partitions x 6 slots
= 768 pixels. One matmul per group:
    lhsT = y-block [128, 128] (6 slots x 21 classes + 2 pad cols, fp8e4)
    rhs  = h-block [128, 144] (6 slots x 24: 22 one-hot cols + ones + zero)
accumulated into one [128, 144] f32 PSUM tile; host sums the 6 diagonal
[21, 24] blocks.

x is 22 classes/pixel (21 real + 1 at -65504). The per-pixel max runs
entirely on DVE in 2x-mode tensor_tensor ops using an overlapped first
level (max(x[0:12], x[10:22])), then 12->6->(2,2)->2, and a final
"swap-max" (max(w2, w2 reversed)) that emits the max duplicated as [m, m]
pairs -- which keeps the is_ge broadcast operand innermost-step-1 so the
one-hot build also runs at 2x. No cross-engine hop inside a tile.

Engines: SP x-loads + out-store | ACT y-loads | GPSIMD h ones-col init |
DVE max tree + is_ge | PE matmuls.

Tiles: [19, 19, 38, 57, 57, 57, 57, 38] groups — small warmup tiles for a
fast pipeline ramp, wider steady-state tiles to amortize DVE per-op
overhead. The first y-loads are deferred until x0 lands so the x stream
(which gates the DVE ramp) gets the full DMA bandwidth at startup.
"""

from contextlib import ExitStack

import ml_dtypes
import numpy as np

import concourse.bass as bass
import concourse.mybir as mybir
from concourse.bass_utils import run_bass_kernel_spmd

B, C, H, W = 8, 21, 512, 512
N = H * W              # 262144 pixels per batch
P = 128                # SBUF partitions
S = 6                  # pixel slots per PE row
CP = 22                # padded class count (21 real + 1 at -65504)
SW = 24                # h slot width: 22 one-hot cols + ones col + zero col
YW = 128               # y group width: 6*21 + 2 pad cols
NG = 342               # total groups per core (342*768 = 262656 >= N)
NPAD = NG * P * S      # padded pixel count
G_TILES = [19, 19, 38, 57, 57, 57, 57, 38]
NT = len(G_TILES)
G_OFF = [sum(G_TILES[:i]) for i in range(NT)]
GMAX = max(G_TILES)
NEG = -65504.0
N_CORES = 8

XW = S * CP            # x cols per group (132)
OC = S * SW            # h cols per group / matmul rhs cols (144)
GSMAX = GMAX * S
_CACHED_NC = None


def build_nc():
    nc = bass.Bass()
    x = nc.declare_dram_parameter("x", [P, NG * XW], mybir.dt.float16, isOutput=False)
    y = nc.declare_dram_parameter("y", [P, NG * YW], mybir.dt.float8e4, isOutput=False)
    out = nc.declare_dram_parameter("out", [P, OC], mybir.dt.float32, isOutput=True)

    with ExitStack() as ctx:
        xs = [
            ctx.enter_context(
                nc.sbuf_tensor(f"xsb{i}", [P, GMAX * XW], mybir.dt.float16)
            )
            for i in range(3)
        ]
        ys = [
            ctx.enter_context(
                nc.sbuf_tensor(f"ysb{i}", [P, GMAX * YW], mybir.dt.float8e4)
            )
            for i in range(2)
        ]
        hs = [
            ctx.enter_context(
                nc.sbuf_tensor(f"hsb{i}", [P, GMAX * OC], mybir.dt.float16)
            )
            for i in range(2)
        ]
        t12 = ctx.enter_context(nc.sbuf_tensor("t12b", [P, GSMAX * 12], mybir.dt.float16))
        u6 = ctx.enter_context(nc.sbuf_tensor("u6b", [P, GSMAX * 6], mybir.dt.float16))
        t2 = ctx.enter_context(nc.sbuf_tensor("t2b", [P, GSMAX * 2], mybir.dt.float16))
        w2 = ctx.enter_context(nc.sbuf_tensor("w2b", [P, GSMAX * 2], mybir.dt.float16))
        md2 = ctx.enter_context(nc.sbuf_tensor("md2b", [P, GSMAX * 2], mybir.dt.float16))
        osb = ctx.enter_context(nc.sbuf_tensor("osb", [P, OC], mybir.dt.float32))
        cm_psum = ctx.enter_context(nc.psum_tensor("cmps", [P, OC], mybir.dt.float32))

        block = ctx.enter_context(nc.Block())
        sxs = [ctx.enter_context(nc.semaphore(f"sx{i}")) for i in range(3)]
        sys_ = [ctx.enter_context(nc.semaphore(f"sy{i}")) for i in range(2)]
        shd = ctx.enter_context(nc.semaphore("shd"))    # DVE isge done, = t+1
        si = ctx.enter_context(nc.semaphore("si"))      # gpsimd init done
        sp = ctx.enter_context(nc.semaphore("sp"))      # PE tile matmuls done, = t+1
        sv2 = ctx.enter_context(nc.semaphore("sv2"))    # final psum copy done
        so = ctx.enter_context(nc.semaphore("so"))      # out DMA done

        @block.sync
        def _(sync):
            for t in range(NT):
                if t >= 3:
                    sync.wait_ge(shd, t - 2)  # isge(t-3) freed x slot
                cols = G_TILES[t] * XW
                sync.dma_start(
                    out=xs[t % 3][:, 0:cols],
                    in_=x[:, G_OFF[t] * XW : G_OFF[t] * XW + cols],
                ).then_inc(sxs[t % 3], 16)
            sync.wait_ge(sv2, 1)
            sync.dma_start(out=out[:], in_=osb[:]).then_inc(so, 16)
            sync.wait_ge(so, 16)

        @block.scalar
        def _(scalar):
            # defer the first y-loads until x0 has landed: x feeds the DVE
            # ramp (the critical path); y isn't consumed until ~7us later.
            scalar.wait_ge(sxs[0], 16)
            for t in range(NT):
                if t >= 2:
                    scalar.wait_ge(sp, t - 1)  # matmul(t-2) freed y slot
                cols = G_TILES[t] * YW
                scalar.dma_start(
                    out=ys[t % 2][:, 0:cols],
                    in_=y[:, G_OFF[t] * YW : G_OFF[t] * YW + cols],
                ).then_inc(sys_[t % 2], 16)

        @block.gpsimd
        def _(gpsimd):
            h30 = hs[0][:].rearrange("p (gs w) -> p gs w", w=SW)
            h31 = hs[1][:].rearrange("p (gs w) -> p gs w", w=SW)
            nc.gpsimd.memset(h30[:, :, 22:23], 1.0)
            nc.gpsimd.memset(h30[:, :, 23:24], 0.0)
            nc.gpsimd.memset(h31[:, :, 22:23], 1.0)
            nc.gpsimd.memset(h31[:, :, 23:24], 0.0).then_inc(si, 1)

        @block.vector
        def _(vector):
            for t in range(NT):
                q = t % 3
                gs = G_TILES[t] * S
                vector.wait_ge(sxs[q], 16 * (t // 3 + 1))
                x3 = xs[q][:].rearrange("p (gs c) -> p gs c", c=CP)[:, 0:gs, :]
                t3 = t12[:].rearrange("p (gs c) -> p gs c", c=12)[:, 0:gs, :]
                u3 = u6[:].rearrange("p (gs c) -> p gs c", c=6)[:, 0:gs, :]
                t23 = t2[:].rearrange("p (gs c) -> p gs c", c=2)[:, 0:gs, :]
                w23 = w2[:].rearrange("p (gs c) -> p gs c", c=2)[:, 0:gs, :]
                m3 = md2[:].rearrange("p (gs c) -> p gs c", c=2)[:, 0:gs, :]
                mx = mybir.AluOpType.max
                nc.vector.tensor_tensor(
                    out=t3, in0=x3[:, :, 0:12], in1=x3[:, :, 10:22], op=mx
                )
                nc.vector.tensor_tensor(
                    out=u3, in0=t3[:, :, 0:6], in1=t3[:, :, 6:12], op=mx
                )
                nc.vector.tensor_tensor(
                    out=t23, in0=u3[:, :, 0:2], in1=u3[:, :, 2:4], op=mx
                )
                nc.vector.tensor_tensor(
                    out=w23, in0=t23, in1=u3[:, :, 4:6], op=mx
                )
                nc.vector.tensor_tensor(
                    out=m3, in0=w23, in1=w23[:, :, ::-1], op=mx
                )
                if t >= 2:
                    vector.wait_ge(sp, t - 1)  # matmul(t-2) freed h slot
                x4 = xs[q][:].rearrange("p (gs c e) -> p gs c e", c=11, e=2)
                h4 = hs[t % 2][:].rearrange("p (gs c e) -> p gs c e", c=12, e=2)
                m4 = (
                    md2[:]
                    .rearrange("p (gs e) -> p gs e", e=2)[:, 0:gs, :]
                    .unsqueeze(2)
                    .to_broadcast((P, gs, 11, 2))
                )
                nc.vector.tensor_tensor(
                    out=h4[:, 0:gs, 0:11, :],
                    in0=x4[:, 0:gs, :, :],
                    in1=m4,
                    op=mybir.AluOpType.is_ge,
                ).then_inc(shd, 1)
            vector.wait_ge(sp, NT)
            nc.vector.tensor_copy(osb[:], cm_psum[:]).then_inc(sv2, 1)

        @block.tensor
        def _(tensor):
            tensor.wait_ge(si, 1)
            for t in range(NT):
                tensor.wait_ge(sys_[t % 2], 16 * (t // 2 + 1))
                tensor.wait_ge(shd, t + 1)
                for g in range(G_TILES[t]):
                    mm = nc.tensor.matmul(
                        out=cm_psum[:],
                        lhsT=ys[t % 2][:, g * YW : (g + 1) * YW],
                        rhs=hs[t % 2][:, g * OC : (g + 1) * OC],
                        start=(t == 0 and g == 0),
                        stop=(t == NT - 1 and g == G_TILES[t] - 1),
                    )
                mm.then_inc(sp, 1)

    return nc


def _get_nc():
    global _CACHED_NC
    if _CACHED_NC is None:
        _CACHED_NC = build_nc()
    return _CACHED_NC


def make_in_maps(input, target):
    inp = np.asarray(input, dtype=np.float32)
    tgt = np.asarray(target, dtype=np.float32)
    in_maps = []
    for b in range(B):
        xb = inp[b].reshape(C, N).T  # [N, C]
        xq = np.full((NPAD, CP), NEG, dtype=np.float16)
        xq[:N, :C] = xb
        # pad pixels keep x = NEG everywhere -> h row junk but y rows are 0
        x_dev = np.ascontiguousarray(
            xq.reshape(NG, S, P, CP).transpose(2, 0, 1, 3)
        ).reshape(P, NG * XW)

        yb = tgt[b].reshape(C, N).T  # [N, C]
        yq = np.zeros((NPAD, C), dtype=np.float32)
        yq[:N] = yb
        y4 = yq.reshape(NG, S, P, C).transpose(2, 0, 1, 3)  # [P,NG,S,C]
        y_dev = np.zeros((P, NG, YW), dtype=ml_dtypes.float8_e4m3)
        y_dev[..., : S * C] = y4.reshape(P, NG, S * C).astype(ml_dtypes.float8_e4m3)
        in_maps.append({"x": x_dev, "y": y_dev.reshape(P, NG * YW)})
    return in_maps


def postprocess(outs):
    final = np.zeros((C, C), dtype=np.float64)
    for o in outs:
        o = np.asarray(o, dtype=np.float64)  # [128, 144]
        cm = np.zeros((C, C), dtype=np.float64)
        rs = np.zeros((C, 1), dtype=np.float64)
        for s in range(S):
            blk = o[s * C : (s + 1) * C, s * SW : (s + 1) * SW]
            cm += blk[:, :C]
            rs[:, 0] += blk[:, 22]
        final += cm / (rs + 1e-8)
    return (final / len(outs)).astype(np.float32)


def kernel(input, target):
    nc = _get_nc()
    in_maps = make_in_maps(input, target)
    res = run_bass_kernel_spmd(nc, in_maps, list(range(N_CORES)))
    return postprocess([r["out"] for r in res.results])

